# revision 39
# baseline (speedup 1.0000x reference)
"""Trainium2 Bass kernel for nn_MultiHeadAttention_48086453846410.

Reference computation (heads folded into the sequence axis, softmax over the
FULL L = seq*heads key axis):
    qp = (q @ wk_w.T + wk_b).reshape(bs, L, d)   # note swapped wk/wq, faithful
    kp = (k @ wq_w.T + wq_b).reshape(bs, L, d)
    vp = (v @ wv_w.T + wv_b).reshape(bs, L, d)
    scores = qp @ kp.T / sqrt(d); attn = softmax(scores, -1)
    o = (attn @ vp).reshape(bs, seq, d*heads)
    out = o @ out_w.T + out_b

Sharding: 8 cores = (batch b in 0..3) x (seq half). Each core owns 256 query
seq positions of one batch (2048 query rows l' = h*256+s). Softmax is over
keys, so query rows are independent -> no collectives.

fp8 DoubleRow strategy (MatmulPerfMode.DoubleRow: both operands fp8e4,
lhsT [K,2,M] / rhs [K,2,N] pairs two 128-contraction blocks in one
instruction at 0.5 cycles per output column -> 4x bf16 FLOP throughput):
 - qp/kp projections: q/k and 64x-prescaled weights quantized to fp8 on
   host; DoubleRow over d-block pairs; ACT epilogue folds the 1/64 and
   bias, writing qpT/kpT directly in fp8 (the scores operands).
 - v projection: 3-term hi/lo fp8 (v8@w8 + vr8@w8 + v8@wr8, residuals
   prepared on host) -> ~bf16 accuracy at 0.75x the bf16 cycle cost.
 - scores: fp8 DoubleRow over d-block pairs (4x).
 - attention softmax trick: ACT computes exp -> fp16, one DVE/Pool
   tensor_scalar pass writes fp8(e-1). The attn matmul consumes (e-1)
   (4x smaller quantized magnitude than e -> 4x less fp8 noise); the
   missing rank-1 term sum_m v[m,:] is added back during normalization
   as a per-partition scalar (vsum, computed on PE with N=1 DoubleRow
   ones-matmuls), and Z = sum(e-1) + 4096.
 - attn@v: vp kept as an fp8 hi/lo pair (vp8 + vpr8 residual, split on
   DVE/Pool from an fp16 intermediate); each chunk-pair runs two
   DoubleRow instructions against the shared (e-1) moving pair -> 2x.
 - out projection stays bf16 (o-side fp8 noise passes undiminished
   through the near-cancelling attention average; measured).
Numpy-simulated end-to-end rel err of this exact datapath: 0.0088
(gate 2e-2; bf16 baseline 0.0027).

Schedule: phase A is merged into phase B so the PE never waits for the
ACT-bound projection epilogues or the weight DMA:
 - A1 (qpT) is split by head pair: slice ls only reads qpT columns for
   heads (2ls, 2ls+1), so each slice's 8 A1 units are emitted one slice
   ahead (slice 0's at the top, slice ls+1's injected mid-slice ls).
 - A2/A3 are g-pipelined with slice 0: scores for head g need only kpT
   tiles 4g..4g+3 and attn needs only vp tiles js=g, so per g we emit
   [A2 x4, A3 x4, scores pairs (g,0),(g,2)] with attn trailing by a
   2-pair skew. Weight DMA streams jr-major to match.

Rejected after measurement: sharing the duplicated kp/vp projections across
the core pair of each batch via pairwise AllGather - a chained-AllGather
microbenchmark on this hardware measured 200-350us per 2MB collective,
so the duplication is cheaper.
"""

import math
import sys

for _p in ("/opt/trn_rl_repo",):
    if _p not in sys.path:
        sys.path.insert(0, _p)

import numpy as np
import ml_dtypes

BS, SEQ, D, HEADS = 4, 512, 512, 8
NCORES = 8
S = SEQ // 2            # 256 query seq rows per core
HD = HEADS * D          # 4096
JT = HD // 128          # 32 tiles of the 4096 projection dim
DT = D // 128           # 4 tiles of the 512 contraction dim
TT = SEQ // 128         # 4 key-seq tiles per head
LSLICES = 4             # l' = 2048 per core, processed in 4 slices of 512
WS = 64.0               # host prescale for fp8 projection weights
NP_BF16 = ml_dtypes.bfloat16
NP_F8 = ml_dtypes.float8_e4m3

_CACHE = {}


def _build_program():
    from concourse import bacc
    import concourse.mybir as mybir
    import concourse.tile as tile
    from concourse.dt import dt

    f32 = dt.float32
    b16 = dt.bfloat16
    f16 = dt.float16
    f8 = dt.float8e4
    Act = mybir.ActivationFunctionType
    Alu = mybir.AluOpType
    DR = mybir.MatmulPerfMode.DoubleRow

    nc = bacc.Bacc(None, target_bir_lowering=False, debug=False,
                   num_devices=NCORES)

    def din(name, shape, dty):
        return nc.dram_tensor(name, shape, dty, kind="ExternalInput").ap()

    qT8 = din("qT8", [D, S], f8)           # q[b, half].T      (d, s)
    kT8 = din("kT8", [D, SEQ], f8)         # k[b].T            (d, t)
    vT8 = din("vT8", [D, SEQ], f8)         # v[b].T hi         (d, t)
    vTr8 = din("vTr8", [D, SEQ], f8)       # v[b].T residual   (d, t)
    wk8T = din("wk8T", [D, HD], f8)        # wk_w.T * 64       (d, j)
    wq8T = din("wq8T", [D, HD], f8)        # wq_w.T * 64       (d, j)
    wv8T = din("wv8T", [D, HD], f8)        # wv_w.T * 64 hi    (d, j)
    wvr8T = din("wvr8T", [D, HD], f8)      # wv_w.T * 64 res   (d, j)
    owT = din("owT", [HD, D], b16)         # out_w.T           (c, r)
    wk_bT = din("wk_bT", [128, JT], f32)   # wk_b.reshape(JT,128).T
    wq_bT = din("wq_bT", [128, JT], f32)
    wv_br = din("wv_br", [128, HD], b16)   # wv_b replicated
    out_br = din("out_br", [128, D], f32)
    ones8 = din("ones8", [128, 256], f8)
    out = nc.dram_tensor("out", [S, D], f32, kind="ExternalOutput").ap()

    inv_sqrt_d = 1.0 / math.sqrt(D)
    inv_ws = 1.0 / WS

    with tile.TileContext(nc) as tc:
        with (
            tc.tile_pool(name="const", bufs=1) as cp,
            tc.tile_pool(name="w8pool", bufs=32) as w8p,
            tc.tile_pool(name="acts", bufs=1) as acp,
            tc.tile_pool(name="state", bufs=1) as sp,
            tc.tile_pool(name="v16p", bufs=4) as v16p,
            tc.tile_pool(name="e16p", bufs=4) as e16p,
            tc.tile_pool(name="e8p", bufs=8) as e8p,
            tc.tile_pool(name="zrp", bufs=2) as zp,
            tc.tile_pool(name="owp", bufs=8) as owp,
            tc.tile_pool(name="psA", bufs=3, space="PSUM") as psA,
            tc.tile_pool(name="psO", bufs=4, space="PSUM") as psO,
            tc.tile_pool(name="psZ", bufs=1, space="PSUM") as psZ,
        ):
            # ---- fp8 weight tiles: one [128, 2048] tile per (nm, jr) ----
            # holds all 4 d-blocks (block-major) for j cols [jr*512, +512).
            # DMA emission is jr-major (the g-pipeline consumption order).
            W8 = {}

            def dma_w8_full(nm, dram, jr, eng):
                t = w8p.tile([128, 2048], f8, tag="w8", name=f"w8_{nm}_{jr}")
                eng.dma_start(
                    out=t.rearrange("p (q n) -> p q n", n=512),
                    in_=dram[:, jr * 512:(jr + 1) * 512].rearrange(
                        "(q p) n -> p q n", p=128))
                W8[nm, jr] = t

            def w8sl(nm, jr, dp, off, width):
                # returns [128, 2, width] lhs/rhs pair view for d-blocks
                # (2dp, 2dp+1), j cols [jr*512+off, +width)
                v = W8[nm, jr].rearrange("p (q n) -> p q n", n=512)
                return v[:, 2 * dp:2 * dp + 2, off:off + width]

            # ---- input DMA: ALL on the dedicated sync queue in need-time
            # priority order. Pool/ACT/DVE sequencers must stay free for
            # epilogue compute (a Pool DMA dispatch costs ~1.1us of queue
            # time and starves the vpr8 subs).
            qT8_sb = acp.tile([128, DT * S], f8, tag="qT8")
            kT8_sb = acp.tile([128, DT * SEQ], f8, tag="kT8")
            vT8_sb = acp.tile([128, DT * SEQ], f8, tag="vT8")
            vTr8_sb = acp.tile([128, DT * SEQ], f8, tag="vTr8")
            wk_bT_sb = cp.tile([128, JT], f32, tag="wkb")
            wq_bT_sb = cp.tile([128, JT], f32, tag="wqb")
            wv_br_sb = cp.tile([128, HD], b16, tag="wvb")
            ones8_sb = cp.tile([128, 256], f8, tag="ones8")
            out_br_sb = cp.tile([128, D], f32, tag="outb")
            ones8_r = ones8_sb.rearrange("p (two n) -> p two n", n=128)

            def dma_wvbr(js):
                # per-head chunk of the replicated v-bias (keeps the 1MB
                # broadcast off the startup critical path)
                nc.sync.dma_start(
                    out=wv_br_sb[:, js * 512:(js + 1) * 512],
                    in_=wv_br[:, js * 512:(js + 1) * 512])

            # qT8 then wk8 jr0 (slice-0 A1 needs heads 0-1 only)
            nc.sync.dma_start(out=qT8_sb.rearrange("p (t n) -> p t n", n=S),
                              in_=qT8.rearrange("(t p) n -> p t n", p=128))
            dma_w8_full("k", wk8T, 0, nc.sync)
            nc.sync.dma_start(out=wk_bT_sb, in_=wk_bT)
            nc.sync.dma_start(out=wq_bT_sb, in_=wq_bT)
            nc.sync.dma_start(out=ones8_sb, in_=ones8)
            dma_w8_full("k", wk8T, 1, nc.sync)
            nc.sync.dma_start(out=kT8_sb.rearrange("p (t n) -> p t n", n=SEQ),
                              in_=kT8.rearrange("(t p) n -> p t n", p=128))
            dma_w8_full("q", wq8T, 0, nc.sync)
            nc.sync.dma_start(out=vT8_sb.rearrange("p (t n) -> p t n", n=SEQ),
                              in_=vT8.rearrange("(t p) n -> p t n", p=128))
            nc.sync.dma_start(
                out=vTr8_sb.rearrange("p (t n) -> p t n", n=SEQ),
                in_=vTr8.rearrange("(t p) n -> p t n", p=128))
            dma_wvbr(0)
            dma_w8_full("v", wv8T, 0, nc.sync)
            dma_w8_full("vr", wvr8T, 0, nc.sync)

            # stream the rest jr-major (g-pipeline consumption order), with
            # wk8 jr 2-7 (A1 for slices 1-3, injected one slice ahead)
            # slotted in when each slice's prefetch needs them.
            for jr in range(1, 8):
                dma_w8_full("q", wq8T, jr, nc.sync)
                dma_w8_full("v", wv8T, jr, nc.sync)
                dma_w8_full("vr", wvr8T, jr, nc.sync)
                dma_wvbr(jr)
                if jr in (2, 4, 6):
                    dma_w8_full("k", wk8T, jr, nc.sync)
                    dma_w8_full("k", wk8T, jr + 1, nc.sync)
            nc.sync.dma_start(out=out_br_sb, in_=out_br)

            # ---- persistent state ----
            # qpT interleaved: col block (dt*HEADS + h)*S
            qpT8_sb = sp.tile([128, JT * S], f8, tag="qpT")       # 8KB/part
            kpT8_sb = sp.tile([128, JT * SEQ], f8, tag="kpT")     # 16KB/part
            vp8_sb = sp.tile([128, TT * HD], f8, tag="vp8")       # 16KB/part
            vpr8_sb = sp.tile([128, TT * HD], f8, tag="vpr8")     # 16KB/part
            oT_sb = sp.tile([128, DT * 2048], b16, tag="oT")      # 16KB/part
            fin32 = sp.tile([128, 2 * D], f32, tag="fin32")       # 4KB/part
            vsum32 = sp.tile([128, DT], f32, tag="vsum32")

            qT8_r = qT8_sb.rearrange("p (t n) -> p t n", n=S)
            kT8_r = kT8_sb.rearrange("p (t n) -> p t n", n=SEQ)
            vT8_r = vT8_sb.rearrange("p (t n) -> p t n", n=SEQ)
            vTr8_r = vTr8_sb.rearrange("p (t n) -> p t n", n=SEQ)
            qpT8_r = qpT8_sb.rearrange("p (t n) -> p t n", n=HEADS * S)
            kpT8_r = kpT8_sb.rearrange("p (j t) -> p j t", t=SEQ)
            vp8_r = vp8_sb.rearrange("p (t c) -> p t c", c=HD)
            vpr8_r = vpr8_sb.rearrange("p (t c) -> p t c", c=HD)

            def emit_a1(jt):
                # qpT[j, s] for j-block jt = (h, dt): fp8 DoubleRow pairs.
                # Epilogue on DVE (tensor_scalar handles scale + per-
                # partition bias) - ACT is the pacing engine in the slices.
                h, dt_of_j = divmod(jt, DT)
                jr, off = divmod(jt * 128, 512)
                ps = psA.tile([128, 512], f32, tag="psA", name=f"a1_{jt}")
                for dp in range(2):
                    nc.tensor.matmul(
                        ps[:, :S],
                        lhsT=w8sl("k", jr, dp, off, 128),
                        rhs=qT8_r[:, 2 * dp:2 * dp + 2, :],
                        start=(dp == 0), stop=(dp == 1), perf_mode=DR)
                nc.vector.tensor_scalar(
                    qpT8_sb[:, (dt_of_j * HEADS + h) * S:
                            (dt_of_j * HEADS + h + 1) * S],
                    ps[:, :S], inv_ws, wk_bT_sb[:, jt:jt + 1],
                    Alu.mult, Alu.add)

            def emit_a2(jt):
                jr, off = divmod(jt * 128, 512)
                ps = psA.tile([128, 512], f32, tag="psA", name=f"a2_{jt}")
                for dp in range(2):
                    nc.tensor.matmul(
                        ps,
                        lhsT=w8sl("q", jr, dp, off, 128),
                        rhs=kT8_r[:, 2 * dp:2 * dp + 2, :],
                        start=(dp == 0), stop=(dp == 1), perf_mode=DR)
                nc.scalar.activation(kpT8_sb[:, jt * SEQ:(jt + 1) * SEQ], ps,
                                     Act.Identity, bias=wq_bT_sb[:, jt:jt + 1],
                                     scale=inv_ws)

            def emit_a3(js, tt):
                # vp[t, j] 3-term fp8: v8@w8 + vr8@w8 + v8@wr8 (64-scaled w)
                ps = psA.tile([128, 512], f32, tag="psA", name=f"a3_{js}_{tt}")
                first = True
                for dp in range(2):
                    for lv, wnm in ((vT8_r, "v"), (vTr8_r, "v"),
                                    (vT8_r, "vr")):
                        nc.tensor.matmul(
                            ps,
                            lhsT=lv[:, 2 * dp:2 * dp + 2,
                                    tt * 128:(tt + 1) * 128],
                            rhs=w8sl(wnm, js, dp, 0, 512),
                            start=first, stop=(dp == 1 and wnm == "vr"),
                            perf_mode=DR)
                        first = False
                c0 = tt * HD + js * 512
                vp16 = v16p.tile([128, 512], f16, tag="v16",
                                 name=f"v16_{js}_{tt}")
                nc.vector.scalar_tensor_tensor(
                    vp16, ps, inv_ws, wv_br_sb[:, js * 512:(js + 1) * 512],
                    Alu.mult, Alu.add)
                nc.vector.tensor_copy(vp8_sb[:, c0:c0 + 512], vp16)
                nc.gpsimd.tensor_sub(vpr8_sb[:, c0:c0 + 512], vp16,
                                     vp8_sb[:, c0:c0 + 512])

            # ---- phase B machinery ----
            prev_outproj = [None]
            ow_cur = [None]
            NPAIRS = HEADS * 2  # 16 chunk-pairs per slice
            SKEW_P = 4

            def prefetch_ow(h0):
                ow_tiles = {}
                for ct in range(h0 * DT, (h0 + 2) * DT):
                    ow_tiles[ct] = owp.tile([128, D], b16, tag="ow",
                                            name=f"ow{ct}")
                    nc.sync.dma_start(out=ow_tiles[ct],
                                      in_=owT[ct * 128:(ct + 1) * 128, :])
                return ow_tiles

            def make_outproj(ls, h0, ow_tiles, final=False):
                # final=True: et-major accumulation (chases the per-et norm
                # writes) + immediate per-half output DMA to cut the tail.
                def outproj():
                    for st in range(2):
                        psc = psA.tile([128, 512], f32, tag="psA",
                                       name=f"psc{ls}_{st}")
                        cts = (range(h0 * DT, (h0 + 2) * DT) if not final
                               else [h * DT + et for et in range(DT)
                                     for h in (h0, h0 + 1)])
                        for ci2, ct in enumerate(cts):
                            h, et = divmod(ct, DT)
                            nc.tensor.matmul(
                                psc,
                                lhsT=oT_sb[:, et * 2048 + h * S + st * 128:
                                           et * 2048 + h * S +
                                           (st + 1) * 128],
                                rhs=ow_tiles[ct],
                                start=(ci2 == 0),
                                stop=(ci2 == 2 * DT - 1))
                        if ls == 0:
                            nc.vector.tensor_add(
                                fin32[:, st * D:(st + 1) * D],
                                psc, out_br_sb)
                        else:
                            nc.vector.tensor_add(
                                fin32[:, st * D:(st + 1) * D],
                                psc, fin32[:, st * D:(st + 1) * D])
                        if final:
                            nc.sync.dma_start(
                                out=out[st * 128:(st + 1) * 128, :],
                                in_=fin32[:, st * D:(st + 1) * D])
                return outproj

            for ls in range(LSLICES):
                h0 = 2 * ls
                po = [psO.tile([128, 512], f32, tag="psO", name=f"po{ls}_{i}")
                      for i in range(DT)]
                psz = psZ.tile([128, 512], f32, tag="psZ", name=f"psz{ls}")
                pending = []

                def emit_attn(pi, g, tp, e8t, po=po, psz=psz):
                    e8pair = e8t.rearrange("p (two n) -> p two n", n=512)
                    # Z first: at the last pair this lets zden/recip start
                    # while the final po matmuls still run
                    nc.tensor.matmul(
                        psz, lhsT=ones8_r, rhs=e8pair,
                        start=(pi == 0), stop=(pi == NPAIRS - 1),
                        perf_mode=DR)
                    for et in range(DT):
                        nc.tensor.matmul(
                            po[et],
                            lhsT=vp8_r[:, tp:tp + 2,
                                       g * 512 + et * 128:
                                       g * 512 + (et + 1) * 128],
                            rhs=e8pair,
                            start=(pi == 0), stop=False, perf_mode=DR)
                        nc.tensor.matmul(
                            po[et],
                            lhsT=vpr8_r[:, tp:tp + 2,
                                        g * 512 + et * 128:
                                        g * 512 + (et + 1) * 128],
                            rhs=e8pair,
                            start=False, stop=(pi == NPAIRS - 1),
                            perf_mode=DR)

                if ls == 0:
                    for jt_h in range(2):          # A1 for heads 0-1
                        for dt_ in range(DT):
                            emit_a1(jt_h * DT + dt_)

                for g in range(HEADS):
                    if ls == 0:
                        # alternate A2/A3 so their ACT/DVE epilogues drain
                        # each other's PSUM tiles without pool stalls
                        for i in range(DT):
                            emit_a2(g * DT + i)
                            emit_a3(g, i)
                    for tp in (0, 2):
                        pi = g * 2 + tp // 2
                        if ls + 1 < LSLICES and 4 <= pi < 12:
                            # one A1 unit of the next slice's head pair per
                            # pair iteration (spreads the DVE epilogues)
                            h_n, dt_n = divmod(pi - 4, DT)
                            emit_a1((2 * (ls + 1) + h_n) * DT + dt_n)
                        if pi == 8:
                            # this slice's out-projection weights (consumed
                            # in the next slice / at the end)
                            ow_cur[0] = prefetch_ow(h0)
                        e8t = e8p.tile([128, 1024], f8, tag="e8",
                                       name=f"e8_{ls}_{pi}")
                        for half in range(2):
                            tt = tp + half
                            ps = psA.tile([128, 512], f32, tag="psA")
                            for dp in range(2):
                                nc.tensor.matmul(
                                    ps,
                                    lhsT=kpT8_r[:, g * DT + 2 * dp:
                                                g * DT + 2 * dp + 2,
                                                tt * 128:(tt + 1) * 128],
                                    rhs=qpT8_r[:, 2 * dp:2 * dp + 2,
                                               h0 * S:(h0 + 2) * S],
                                    start=(dp == 0), stop=(dp == 1),
                                    perf_mode=DR)
                            ex16 = e16p.tile([128, 512], f16, tag="e16")
                            nc.scalar.activation(ex16, ps, Act.Exp,
                                                 bias=0.0, scale=inv_sqrt_d)
                            nc.vector.tensor_scalar(
                                e8t[:, half * 512:(half + 1) * 512],
                                ex16, 1.0, None, Alu.subtract)
                        pending.append((pi, g, tp, e8t))
                        if (ls > 0 and pi == 3
                                and prev_outproj[0] is not None):
                            prev_outproj[0]()
                            prev_outproj[0] = None
                        if len(pending) > SKEW_P:
                            emit_attn(*pending.pop(0))

                if ls == 0:
                    # vsum[e] = sum_m (vp8+vpr8)[m, e] via N=1 DoubleRow
                    vsumP = psA.tile([128, DT], f32, tag="psA", name="vsumP")
                    for et in range(DT):
                        n_mm = 2 * HEADS * 2
                        i = 0
                        for g2 in range(HEADS):
                            for tp2 in (0, 2):
                                for src in (vp8_r, vpr8_r):
                                    nc.tensor.matmul(
                                        vsumP[:, et:et + 1],
                                        lhsT=src[:, tp2:tp2 + 2,
                                                 g2 * 512 + et * 128:
                                                 g2 * 512 + (et + 1) * 128],
                                        rhs=ones8_r[:, :, 0:1],
                                        start=(i == 0), stop=(i == n_mm - 1),
                                        perf_mode=DR)
                                    i += 1
                    nc.vector.tensor_copy(vsum32, vsumP)

                for args in pending:
                    emit_attn(*args)

                # Z finalization + normalization; the rank-1 vsum correction
                # and the 1/Z scaling fuse into one DVE op per et.
                zden = zp.tile([128, 512], f32, tag="zden", name=f"zden{ls}")
                nc.vector.tensor_scalar(zden, psz, float(HD), None, Alu.add)
                zr = zp.tile([128, 512], f32, tag="zr", name=f"zr{ls}")
                nc.vector.reciprocal(zr, zden)
                for et in range(DT):
                    nc.vector.scalar_tensor_tensor(
                        oT_sb[:, et * 2048 + ls * 512:
                              et * 2048 + ls * 512 + 512],
                        po[et], vsum32[:, et:et + 1], zr,
                        Alu.add, Alu.mult)

                if prev_outproj[0] is not None:
                    prev_outproj[0]()
                prev_outproj[0] = make_outproj(ls, h0, ow_cur[0],
                                               final=(ls == LSLICES - 1))

            prev_outproj[0]()

    nc.compile()
    return nc


def _get_program():
    if "nc" not in _CACHE:
        _CACHE["nc"] = _build_program()
    return _CACHE["nc"]


def _q8(x):
    return np.asarray(x, np.float32).astype(NP_F8)


def _q8r(x):
    """hi/lo fp8 split of x: returns (hi, lo) with hi + lo ~= x."""
    x = np.asarray(x, np.float32)
    hi = x.astype(NP_F8)
    lo = (x - hi.astype(np.float32)).astype(NP_F8)
    return hi, lo


def _prep_shared(inputs):
    f32c = np.ascontiguousarray

    def t32(x):
        return f32c(np.asarray(x, np.float32).T)

    wv8, wvr8 = _q8r(t32(inputs["wv_w"]) * np.float32(WS))
    shared = {
        "wk8T": _q8(t32(inputs["wk_w"]) * np.float32(WS)),
        "wq8T": _q8(t32(inputs["wq_w"]) * np.float32(WS)),
        "wv8T": f32c(wv8),
        "wvr8T": f32c(wvr8),
        "owT": t32(inputs["out_w"]).astype(NP_BF16),
        "wk_bT": f32c(np.asarray(inputs["wk_b"], np.float32).reshape(JT, 128).T),
        "wq_bT": f32c(np.asarray(inputs["wq_b"], np.float32).reshape(JT, 128).T),
        "wv_br": f32c(np.broadcast_to(
            np.asarray(inputs["wv_b"], np.float32)[None, :],
            (128, HD))).astype(NP_BF16),
        "out_br": f32c(np.broadcast_to(
            np.asarray(inputs["out_b"], np.float32)[None, :], (128, D))),
        "ones8": np.ones((128, 256), NP_F8),
    }
    return shared


def _make_in_maps(inputs):
    shared = _prep_shared(inputs)
    q = np.asarray(inputs["q"], np.float32)
    k = np.asarray(inputs["k"], np.float32)
    v = np.asarray(inputs["v"], np.float32)
    in_maps = []
    for core in range(NCORES):
        b, half = divmod(core, 2)
        m = dict(shared)
        m["qT8"] = _q8(np.ascontiguousarray(
            q[b, half * S:(half + 1) * S, :].T))
        m["kT8"] = _q8(np.ascontiguousarray(k[b].T))
        v8, vr8 = _q8r(np.ascontiguousarray(v[b].T))
        m["vT8"] = np.ascontiguousarray(v8)
        m["vTr8"] = np.ascontiguousarray(vr8)
        in_maps.append(m)
    return in_maps


def kernel(**inputs):
    from concourse.bass_utils import run_bass_kernel_spmd

    nc = _get_program()
    in_maps = _make_in_maps(inputs)
    res = run_bass_kernel_spmd(nc, in_maps, core_ids=list(range(NCORES)))
    _CACHE["last_results"] = res
    out = np.empty((BS, SEQ, D), np.float32)
    for core in range(NCORES):
        b, half = divmod(core, 2)
        out[b, half * S:(half + 1) * S, :] = res.results[core]["out"]
    return out


if __name__ == "__main__":
    rng = np.random.default_rng(0)
    fake = {
        "q": rng.standard_normal((BS, SEQ, D)).astype(np.float32),
        "k": rng.standard_normal((BS, SEQ, D)).astype(np.float32),
        "v": rng.standard_normal((BS, SEQ, D)).astype(np.float32),
        "wq_w": (rng.standard_normal((D * HEADS, D)) * 0.02).astype(np.float32),
        "wq_b": (rng.standard_normal((D * HEADS,)) * 0.02).astype(np.float32),
        "wk_w": (rng.standard_normal((D * HEADS, D)) * 0.02).astype(np.float32),
        "wk_b": (rng.standard_normal((D * HEADS,)) * 0.02).astype(np.float32),
        "wv_w": (rng.standard_normal((D * HEADS, D)) * 0.02).astype(np.float32),
        "wv_b": (rng.standard_normal((D * HEADS,)) * 0.02).astype(np.float32),
        "out_w": (rng.standard_normal((D, D * HEADS)) * 0.02).astype(np.float32),
        "out_b": (rng.standard_normal((D,)) * 0.02).astype(np.float32),
    }
    o = kernel(**fake)
    print("kernel ran, out shape", o.shape, "std", o.std())


# revision 40
# speedup vs baseline: 1.0180x; 1.0180x over previous
"""Trainium2 Bass kernel for nn_MultiHeadAttention_48086453846410.

Reference computation (heads folded into the sequence axis, softmax over the
FULL L = seq*heads key axis):
    qp = (q @ wk_w.T + wk_b).reshape(bs, L, d)   # note swapped wk/wq, faithful
    kp = (k @ wq_w.T + wq_b).reshape(bs, L, d)
    vp = (v @ wv_w.T + wv_b).reshape(bs, L, d)
    scores = qp @ kp.T / sqrt(d); attn = softmax(scores, -1)
    o = (attn @ vp).reshape(bs, seq, d*heads)
    out = o @ out_w.T + out_b

Sharding: 8 cores = (batch b in 0..3) x (seq half). Each core owns 256 query
seq positions of one batch (2048 query rows l' = h*256+s). Softmax is over
keys, so query rows are independent -> no collectives.

fp8 DoubleRow strategy (MatmulPerfMode.DoubleRow: both operands fp8e4,
lhsT [K,2,M] / rhs [K,2,N] pairs two 128-contraction blocks in one
instruction at 0.5 cycles per output column -> 4x bf16 FLOP throughput):
 - qp/kp projections: q/k and 64x-prescaled weights quantized to fp8 on
   host; DoubleRow over d-block pairs; ACT epilogue folds the 1/64 and
   bias, writing qpT/kpT directly in fp8 (the scores operands).
 - v projection: 3-term hi/lo fp8 (v8@w8 + vr8@w8 + v8@wr8, residuals
   prepared on host) -> ~bf16 accuracy at 0.75x the bf16 cycle cost.
 - scores: fp8 DoubleRow over d-block pairs (4x).
 - attention softmax trick: ACT computes exp -> fp16, one DVE/Pool
   tensor_scalar pass writes fp8(e-1). The attn matmul consumes (e-1)
   (4x smaller quantized magnitude than e -> 4x less fp8 noise); the
   missing rank-1 term sum_m v[m,:] is added back during normalization
   as a per-partition scalar (vsum, computed on PE with N=1 DoubleRow
   ones-matmuls), and Z = sum(e-1) + 4096.
 - attn@v: vp kept as an fp8 hi/lo pair (vp8 + vpr8 residual, split on
   DVE/Pool from an fp16 intermediate); each chunk-pair runs two
   DoubleRow instructions against the shared (e-1) moving pair -> 2x.
 - out projection stays bf16 (o-side fp8 noise passes undiminished
   through the near-cancelling attention average; measured).
Numpy-simulated end-to-end rel err of this exact datapath: 0.0088
(gate 2e-2; bf16 baseline 0.0027).

Schedule: phase A is merged into phase B so the PE never waits for the
ACT-bound projection epilogues or the weight DMA:
 - A1 (qpT) is split by head pair: slice ls only reads qpT columns for
   heads (2ls, 2ls+1), so each slice's 8 A1 units are emitted one slice
   ahead (slice 0's at the top, slice ls+1's injected mid-slice ls).
 - A2/A3 are g-pipelined with slice 0: scores for head g need only kpT
   tiles 4g..4g+3 and attn needs only vp tiles js=g, so per g we emit
   [A2 x4, A3 x4, scores pairs (g,0),(g,2)] with attn trailing by a
   2-pair skew. Weight DMA streams jr-major to match.

Rejected after measurement: sharing the duplicated kp/vp projections across
the core pair of each batch via pairwise AllGather - a chained-AllGather
microbenchmark on this hardware measured 200-350us per 2MB collective,
so the duplication is cheaper.
"""

import math
import sys

for _p in ("/opt/trn_rl_repo",):
    if _p not in sys.path:
        sys.path.insert(0, _p)

import numpy as np
import ml_dtypes

BS, SEQ, D, HEADS = 4, 512, 512, 8
NCORES = 8
S = SEQ // 2            # 256 query seq rows per core
HD = HEADS * D          # 4096
JT = HD // 128          # 32 tiles of the 4096 projection dim
DT = D // 128           # 4 tiles of the 512 contraction dim
TT = SEQ // 128         # 4 key-seq tiles per head
LSLICES = 4             # l' = 2048 per core, processed in 4 slices of 512
WS = 64.0               # host prescale for fp8 projection weights
NP_BF16 = ml_dtypes.bfloat16
NP_F8 = ml_dtypes.float8_e4m3

_CACHE = {}


def _build_program():
    from concourse import bacc
    import concourse.mybir as mybir
    import concourse.tile as tile
    from concourse.dt import dt

    f32 = dt.float32
    b16 = dt.bfloat16
    f16 = dt.float16
    f8 = dt.float8e4
    Act = mybir.ActivationFunctionType
    Alu = mybir.AluOpType
    DR = mybir.MatmulPerfMode.DoubleRow

    nc = bacc.Bacc(None, target_bir_lowering=False, debug=False,
                   num_devices=NCORES)

    def din(name, shape, dty):
        return nc.dram_tensor(name, shape, dty, kind="ExternalInput").ap()

    qT8 = din("qT8", [D, S], f8)           # q[b, half].T      (d, s)
    kT8 = din("kT8", [D, SEQ], f8)         # k[b].T            (d, t)
    vT8 = din("vT8", [D, SEQ], f8)         # v[b].T hi         (d, t)
    vTr8 = din("vTr8", [D, SEQ], f8)       # v[b].T residual   (d, t)
    wk8T = din("wk8T", [D, HD], f8)        # wk_w.T * 64       (d, j)
    wq8T = din("wq8T", [D, HD], f8)        # wq_w.T * 64       (d, j)
    wv8T = din("wv8T", [D, HD], f8)        # wv_w.T * 64 hi    (d, j)
    wvr8T = din("wvr8T", [D, HD], f8)      # wv_w.T * 64 res   (d, j)
    owT = din("owT", [HD, D], b16)         # out_w.T           (c, r)
    wk_bT = din("wk_bT", [128, JT], f32)   # wk_b.reshape(JT,128).T
    wq_bT = din("wq_bT", [128, JT], f32)
    wv_br = din("wv_br", [128, HD], b16)   # wv_b replicated
    out_br = din("out_br", [128, D], f32)
    ones8 = din("ones8", [128, 256], f8)
    out = nc.dram_tensor("out", [S, D], f32, kind="ExternalOutput").ap()

    inv_sqrt_d = 1.0 / math.sqrt(D)
    inv_ws = 1.0 / WS

    with tile.TileContext(nc) as tc:
        with (
            tc.tile_pool(name="const", bufs=1) as cp,
            tc.tile_pool(name="w8pool", bufs=32) as w8p,
            tc.tile_pool(name="acts", bufs=1) as acp,
            tc.tile_pool(name="state", bufs=1) as sp,
            tc.tile_pool(name="v16p", bufs=4) as v16p,
            tc.tile_pool(name="e16p", bufs=4) as e16p,
            tc.tile_pool(name="e8p", bufs=8) as e8p,
            tc.tile_pool(name="zrp", bufs=2) as zp,
            tc.tile_pool(name="owp", bufs=8) as owp,
            tc.tile_pool(name="psA", bufs=3, space="PSUM") as psA,
            tc.tile_pool(name="psO", bufs=4, space="PSUM") as psO,
            tc.tile_pool(name="psZ", bufs=1, space="PSUM") as psZ,
        ):
            # ---- fp8 weight tiles: one [128, 2048] tile per (nm, jr) ----
            # holds all 4 d-blocks (block-major) for j cols [jr*512, +512).
            # DMA emission is jr-major (the g-pipeline consumption order).
            W8 = {}

            def dma_w8_full(nm, dram, jr, eng):
                t = w8p.tile([128, 2048], f8, tag="w8", name=f"w8_{nm}_{jr}")
                eng.dma_start(
                    out=t.rearrange("p (q n) -> p q n", n=512),
                    in_=dram[:, jr * 512:(jr + 1) * 512].rearrange(
                        "(q p) n -> p q n", p=128))
                W8[nm, jr] = t

            def w8sl(nm, jr, dp, off, width):
                # returns [128, 2, width] lhs/rhs pair view for d-blocks
                # (2dp, 2dp+1), j cols [jr*512+off, +width)
                v = W8[nm, jr].rearrange("p (q n) -> p q n", n=512)
                return v[:, 2 * dp:2 * dp + 2, off:off + width]

            # ---- input DMA: ALL on the dedicated sync queue in need-time
            # priority order. Pool/ACT/DVE sequencers must stay free for
            # epilogue compute (a Pool DMA dispatch costs ~1.1us of queue
            # time and starves the vpr8 subs).
            qT8_sb = acp.tile([128, DT * S], f8, tag="qT8")
            kT8_sb = acp.tile([128, DT * SEQ], f8, tag="kT8")
            vT8_sb = acp.tile([128, DT * SEQ], f8, tag="vT8")
            vTr8_sb = acp.tile([128, DT * SEQ], f8, tag="vTr8")
            wk_bT_sb = cp.tile([128, JT], f32, tag="wkb")
            wq_bT_sb = cp.tile([128, JT], f32, tag="wqb")
            wv_br_sb = cp.tile([128, HD], b16, tag="wvb")
            ones8_sb = cp.tile([128, 256], f8, tag="ones8")
            out_br_sb = cp.tile([128, D], f32, tag="outb")
            ones8_r = ones8_sb.rearrange("p (two n) -> p two n", n=128)

            def dma_wvbr(js):
                # per-head chunk of the replicated v-bias (keeps the 1MB
                # broadcast off the startup critical path)
                nc.sync.dma_start(
                    out=wv_br_sb[:, js * 512:(js + 1) * 512],
                    in_=wv_br[:, js * 512:(js + 1) * 512])

            # qT8 then wk8 jr0 (slice-0 A1 needs heads 0-1 only)
            nc.sync.dma_start(out=qT8_sb.rearrange("p (t n) -> p t n", n=S),
                              in_=qT8.rearrange("(t p) n -> p t n", p=128))
            dma_w8_full("k", wk8T, 0, nc.sync)
            nc.sync.dma_start(out=wk_bT_sb, in_=wk_bT)
            nc.sync.dma_start(out=wq_bT_sb, in_=wq_bT)
            nc.sync.dma_start(out=ones8_sb, in_=ones8)
            dma_w8_full("k", wk8T, 1, nc.sync)
            nc.sync.dma_start(out=kT8_sb.rearrange("p (t n) -> p t n", n=SEQ),
                              in_=kT8.rearrange("(t p) n -> p t n", p=128))
            dma_w8_full("q", wq8T, 0, nc.sync)
            nc.sync.dma_start(out=vT8_sb.rearrange("p (t n) -> p t n", n=SEQ),
                              in_=vT8.rearrange("(t p) n -> p t n", p=128))
            nc.sync.dma_start(
                out=vTr8_sb.rearrange("p (t n) -> p t n", n=SEQ),
                in_=vTr8.rearrange("(t p) n -> p t n", p=128))
            dma_wvbr(0)
            dma_w8_full("v", wv8T, 0, nc.sync)
            dma_w8_full("vr", wvr8T, 0, nc.sync)

            # stream the rest jr-major (g-pipeline consumption order), with
            # wk8 jr 2-7 (A1 for slices 1-3, injected one slice ahead)
            # slotted in when each slice's prefetch needs them.
            for jr in range(1, 8):
                dma_w8_full("q", wq8T, jr, nc.sync)
                dma_w8_full("v", wv8T, jr, nc.sync)
                dma_w8_full("vr", wvr8T, jr, nc.sync)
                dma_wvbr(jr)
                if jr in (2, 4, 6):
                    dma_w8_full("k", wk8T, jr, nc.sync)
                    dma_w8_full("k", wk8T, jr + 1, nc.sync)
            nc.sync.dma_start(out=out_br_sb, in_=out_br)

            # ---- persistent state ----
            # qpT interleaved: col block (dt*HEADS + h)*S
            qpT8_sb = sp.tile([128, JT * S], f8, tag="qpT")       # 8KB/part
            kpT8_sb = sp.tile([128, JT * SEQ], f8, tag="kpT")     # 16KB/part
            vp8_sb = sp.tile([128, TT * HD], f8, tag="vp8")       # 16KB/part
            vpr8_sb = sp.tile([128, TT * HD], f8, tag="vpr8")     # 16KB/part
            oT_sb = sp.tile([128, DT * 2048], b16, tag="oT")      # 16KB/part
            fin32 = sp.tile([128, 2 * D], f32, tag="fin32")       # 4KB/part
            vsum32 = sp.tile([128, DT], f32, tag="vsum32")

            qT8_r = qT8_sb.rearrange("p (t n) -> p t n", n=S)
            kT8_r = kT8_sb.rearrange("p (t n) -> p t n", n=SEQ)
            vT8_r = vT8_sb.rearrange("p (t n) -> p t n", n=SEQ)
            vTr8_r = vTr8_sb.rearrange("p (t n) -> p t n", n=SEQ)
            qpT8_r = qpT8_sb.rearrange("p (t n) -> p t n", n=HEADS * S)
            kpT8_r = kpT8_sb.rearrange("p (j t) -> p j t", t=SEQ)
            vp8_r = vp8_sb.rearrange("p (t c) -> p t c", c=HD)
            vpr8_r = vpr8_sb.rearrange("p (t c) -> p t c", c=HD)

            def emit_a1(jt):
                # qpT[j, s] for j-block jt = (h, dt): fp8 DoubleRow pairs.
                # Epilogue on DVE (tensor_scalar handles scale + per-
                # partition bias) - ACT is the pacing engine in the slices.
                h, dt_of_j = divmod(jt, DT)
                jr, off = divmod(jt * 128, 512)
                ps = psA.tile([128, 512], f32, tag="psA", name=f"a1_{jt}")
                for dp in range(2):
                    nc.tensor.matmul(
                        ps[:, :S],
                        lhsT=w8sl("k", jr, dp, off, 128),
                        rhs=qT8_r[:, 2 * dp:2 * dp + 2, :],
                        start=(dp == 0), stop=(dp == 1), perf_mode=DR)
                nc.vector.tensor_scalar(
                    qpT8_sb[:, (dt_of_j * HEADS + h) * S:
                            (dt_of_j * HEADS + h + 1) * S],
                    ps[:, :S], inv_ws, wk_bT_sb[:, jt:jt + 1],
                    Alu.mult, Alu.add)

            def emit_a2(jt):
                jr, off = divmod(jt * 128, 512)
                ps = psA.tile([128, 512], f32, tag="psA", name=f"a2_{jt}")
                for dp in range(2):
                    nc.tensor.matmul(
                        ps,
                        lhsT=w8sl("q", jr, dp, off, 128),
                        rhs=kT8_r[:, 2 * dp:2 * dp + 2, :],
                        start=(dp == 0), stop=(dp == 1), perf_mode=DR)
                nc.scalar.activation(kpT8_sb[:, jt * SEQ:(jt + 1) * SEQ], ps,
                                     Act.Identity, bias=wq_bT_sb[:, jt:jt + 1],
                                     scale=inv_ws)

            def emit_a3(js, tt):
                # vp[t, j] 3-term fp8: v8@w8 + vr8@w8 + v8@wr8 (64-scaled w)
                ps = psA.tile([128, 512], f32, tag="psA", name=f"a3_{js}_{tt}")
                first = True
                for dp in range(2):
                    for lv, wnm in ((vT8_r, "v"), (vTr8_r, "v"),
                                    (vT8_r, "vr")):
                        nc.tensor.matmul(
                            ps,
                            lhsT=lv[:, 2 * dp:2 * dp + 2,
                                    tt * 128:(tt + 1) * 128],
                            rhs=w8sl(wnm, js, dp, 0, 512),
                            start=first, stop=(dp == 1 and wnm == "vr"),
                            perf_mode=DR)
                        first = False
                c0 = tt * HD + js * 512
                vp16 = v16p.tile([128, 512], f16, tag="v16",
                                 name=f"v16_{js}_{tt}")
                nc.vector.scalar_tensor_tensor(
                    vp16, ps, inv_ws, wv_br_sb[:, js * 512:(js + 1) * 512],
                    Alu.mult, Alu.add)
                nc.vector.tensor_copy(vp8_sb[:, c0:c0 + 512], vp16)
                nc.gpsimd.tensor_sub(vpr8_sb[:, c0:c0 + 512], vp16,
                                     vp8_sb[:, c0:c0 + 512])

            # ---- phase B machinery ----
            prev_outproj = [None]
            ow_cur = [None]
            NPAIRS = HEADS * 2  # 16 chunk-pairs per slice
            SKEW_P = 4

            def prefetch_ow(h0):
                ow_tiles = {}
                for ct in range(h0 * DT, (h0 + 2) * DT):
                    ow_tiles[ct] = owp.tile([128, D], b16, tag="ow",
                                            name=f"ow{ct}")
                    nc.sync.dma_start(out=ow_tiles[ct],
                                      in_=owT[ct * 128:(ct + 1) * 128, :])
                return ow_tiles

            def make_outproj(ls, h0, ow_tiles, final=False):
                # final=True: et-major accumulation (chases the per-et norm
                # writes) + immediate per-half output DMA to cut the tail.
                def outproj():
                    for st in range(2):
                        psc = psA.tile([128, 512], f32, tag="psA",
                                       name=f"psc{ls}_{st}")
                        cts = (range(h0 * DT, (h0 + 2) * DT) if not final
                               else [h * DT + et for et in range(DT)
                                     for h in (h0, h0 + 1)])
                        for ci2, ct in enumerate(cts):
                            h, et = divmod(ct, DT)
                            nc.tensor.matmul(
                                psc,
                                lhsT=oT_sb[:, et * 2048 + h * S + st * 128:
                                           et * 2048 + h * S +
                                           (st + 1) * 128],
                                rhs=ow_tiles[ct],
                                start=(ci2 == 0),
                                stop=(ci2 == 2 * DT - 1))
                        if ls == 0:
                            nc.vector.tensor_add(
                                fin32[:, st * D:(st + 1) * D],
                                psc, out_br_sb)
                        else:
                            nc.vector.tensor_add(
                                fin32[:, st * D:(st + 1) * D],
                                psc, fin32[:, st * D:(st + 1) * D])
                        if final:
                            nc.sync.dma_start(
                                out=out[st * 128:(st + 1) * 128, :],
                                in_=fin32[:, st * D:(st + 1) * D])
                return outproj

            for ls in range(LSLICES):
                h0 = 2 * ls
                po = [psO.tile([128, 512], f32, tag="psO", name=f"po{ls}_{i}")
                      for i in range(DT)]
                psz = psZ.tile([128, 512], f32, tag="psZ", name=f"psz{ls}")
                pending = []

                def emit_attn(pi, g, tp, e8t, po=po, psz=psz):
                    e8pair = e8t.rearrange("p (two n) -> p two n", n=512)
                    # Z first: at the last pair this lets zden/recip start
                    # while the final po matmuls still run
                    nc.tensor.matmul(
                        psz, lhsT=ones8_r, rhs=e8pair,
                        start=(pi == 0), stop=(pi == NPAIRS - 1),
                        perf_mode=DR)
                    for et in range(DT):
                        nc.tensor.matmul(
                            po[et],
                            lhsT=vp8_r[:, tp:tp + 2,
                                       g * 512 + et * 128:
                                       g * 512 + (et + 1) * 128],
                            rhs=e8pair,
                            start=(pi == 0), stop=False, perf_mode=DR)
                        nc.tensor.matmul(
                            po[et],
                            lhsT=vpr8_r[:, tp:tp + 2,
                                        g * 512 + et * 128:
                                        g * 512 + (et + 1) * 128],
                            rhs=e8pair,
                            start=False, stop=(pi == NPAIRS - 1),
                            perf_mode=DR)

                if ls == 0:
                    for jt_h in range(2):          # A1 for heads 0-1
                        for dt_ in range(DT):
                            emit_a1(jt_h * DT + dt_)

                for g in range(HEADS):
                    if ls == 0:
                        # alternate A2/A3 so their ACT/DVE epilogues drain
                        # each other's PSUM tiles without pool stalls
                        for i in range(DT):
                            emit_a2(g * DT + i)
                            emit_a3(g, i)
                    for tp in (0, 2):
                        pi = g * 2 + tp // 2
                        if ls + 1 < LSLICES and 4 <= pi < 12:
                            # one A1 unit of the next slice's head pair per
                            # pair iteration (spreads the DVE epilogues)
                            h_n, dt_n = divmod(pi - 4, DT)
                            emit_a1((2 * (ls + 1) + h_n) * DT + dt_n)
                        if pi == 8:
                            # this slice's out-projection weights (consumed
                            # in the next slice / at the end)
                            ow_cur[0] = prefetch_ow(h0)
                        e8t = e8p.tile([128, 1024], f8, tag="e8",
                                       name=f"e8_{ls}_{pi}")
                        for half in range(2):
                            tt = tp + half
                            ps = psA.tile([128, 512], f32, tag="psA")
                            for dp in range(2):
                                nc.tensor.matmul(
                                    ps,
                                    lhsT=kpT8_r[:, g * DT + 2 * dp:
                                                g * DT + 2 * dp + 2,
                                                tt * 128:(tt + 1) * 128],
                                    rhs=qpT8_r[:, 2 * dp:2 * dp + 2,
                                               h0 * S:(h0 + 2) * S],
                                    start=(dp == 0), stop=(dp == 1),
                                    perf_mode=DR)
                            ex16 = e16p.tile([128, 512], f16, tag="e16")
                            nc.scalar.activation(ex16, ps, Act.Exp,
                                                 bias=0.0, scale=inv_sqrt_d)
                            nc.vector.tensor_scalar(
                                e8t[:, half * 512:(half + 1) * 512],
                                ex16, 1.0, None, Alu.subtract)
                        pending.append((pi, g, tp, e8t))
                        if (ls > 0 and pi == 1
                                and prev_outproj[0] is not None):
                            prev_outproj[0]()
                            prev_outproj[0] = None
                        if len(pending) > SKEW_P:
                            emit_attn(*pending.pop(0))

                if ls == 0:
                    # vsum[e] = sum_m (vp8+vpr8)[m, e] via N=1 DoubleRow
                    vsumP = psA.tile([128, DT], f32, tag="psA", name="vsumP")
                    for et in range(DT):
                        n_mm = 2 * HEADS * 2
                        i = 0
                        for g2 in range(HEADS):
                            for tp2 in (0, 2):
                                for src in (vp8_r, vpr8_r):
                                    nc.tensor.matmul(
                                        vsumP[:, et:et + 1],
                                        lhsT=src[:, tp2:tp2 + 2,
                                                 g2 * 512 + et * 128:
                                                 g2 * 512 + (et + 1) * 128],
                                        rhs=ones8_r[:, :, 0:1],
                                        start=(i == 0), stop=(i == n_mm - 1),
                                        perf_mode=DR)
                                    i += 1
                    nc.vector.tensor_copy(vsum32, vsumP)

                for args in pending:
                    emit_attn(*args)

                # Z finalization + normalization; the rank-1 vsum correction
                # and the 1/Z scaling fuse into one DVE op per et.
                zden = zp.tile([128, 512], f32, tag="zden", name=f"zden{ls}")
                nc.vector.tensor_scalar(zden, psz, float(HD), None, Alu.add)
                zr = zp.tile([128, 512], f32, tag="zr", name=f"zr{ls}")
                nc.vector.reciprocal(zr, zden)
                for et in range(DT):
                    nc.vector.scalar_tensor_tensor(
                        oT_sb[:, et * 2048 + ls * 512:
                              et * 2048 + ls * 512 + 512],
                        po[et], vsum32[:, et:et + 1], zr,
                        Alu.add, Alu.mult)

                if prev_outproj[0] is not None:
                    prev_outproj[0]()
                prev_outproj[0] = make_outproj(ls, h0, ow_cur[0],
                                               final=(ls == LSLICES - 1))

            prev_outproj[0]()

    nc.compile()
    return nc


def _get_program():
    if "nc" not in _CACHE:
        _CACHE["nc"] = _build_program()
    return _CACHE["nc"]


def _q8(x):
    return np.asarray(x, np.float32).astype(NP_F8)


def _q8r(x):
    """hi/lo fp8 split of x: returns (hi, lo) with hi + lo ~= x."""
    x = np.asarray(x, np.float32)
    hi = x.astype(NP_F8)
    lo = (x - hi.astype(np.float32)).astype(NP_F8)
    return hi, lo


def _prep_shared(inputs):
    f32c = np.ascontiguousarray

    def t32(x):
        return f32c(np.asarray(x, np.float32).T)

    wv8, wvr8 = _q8r(t32(inputs["wv_w"]) * np.float32(WS))
    shared = {
        "wk8T": _q8(t32(inputs["wk_w"]) * np.float32(WS)),
        "wq8T": _q8(t32(inputs["wq_w"]) * np.float32(WS)),
        "wv8T": f32c(wv8),
        "wvr8T": f32c(wvr8),
        "owT": t32(inputs["out_w"]).astype(NP_BF16),
        "wk_bT": f32c(np.asarray(inputs["wk_b"], np.float32).reshape(JT, 128).T),
        "wq_bT": f32c(np.asarray(inputs["wq_b"], np.float32).reshape(JT, 128).T),
        "wv_br": f32c(np.broadcast_to(
            np.asarray(inputs["wv_b"], np.float32)[None, :],
            (128, HD))).astype(NP_BF16),
        "out_br": f32c(np.broadcast_to(
            np.asarray(inputs["out_b"], np.float32)[None, :], (128, D))),
        "ones8": np.ones((128, 256), NP_F8),
    }
    return shared


def _make_in_maps(inputs):
    shared = _prep_shared(inputs)
    q = np.asarray(inputs["q"], np.float32)
    k = np.asarray(inputs["k"], np.float32)
    v = np.asarray(inputs["v"], np.float32)
    in_maps = []
    for core in range(NCORES):
        b, half = divmod(core, 2)
        m = dict(shared)
        m["qT8"] = _q8(np.ascontiguousarray(
            q[b, half * S:(half + 1) * S, :].T))
        m["kT8"] = _q8(np.ascontiguousarray(k[b].T))
        v8, vr8 = _q8r(np.ascontiguousarray(v[b].T))
        m["vT8"] = np.ascontiguousarray(v8)
        m["vTr8"] = np.ascontiguousarray(vr8)
        in_maps.append(m)
    return in_maps


def kernel(**inputs):
    from concourse.bass_utils import run_bass_kernel_spmd

    nc = _get_program()
    in_maps = _make_in_maps(inputs)
    res = run_bass_kernel_spmd(nc, in_maps, core_ids=list(range(NCORES)))
    _CACHE["last_results"] = res
    out = np.empty((BS, SEQ, D), np.float32)
    for core in range(NCORES):
        b, half = divmod(core, 2)
        out[b, half * S:(half + 1) * S, :] = res.results[core]["out"]
    return out


if __name__ == "__main__":
    rng = np.random.default_rng(0)
    fake = {
        "q": rng.standard_normal((BS, SEQ, D)).astype(np.float32),
        "k": rng.standard_normal((BS, SEQ, D)).astype(np.float32),
        "v": rng.standard_normal((BS, SEQ, D)).astype(np.float32),
        "wq_w": (rng.standard_normal((D * HEADS, D)) * 0.02).astype(np.float32),
        "wq_b": (rng.standard_normal((D * HEADS,)) * 0.02).astype(np.float32),
        "wk_w": (rng.standard_normal((D * HEADS, D)) * 0.02).astype(np.float32),
        "wk_b": (rng.standard_normal((D * HEADS,)) * 0.02).astype(np.float32),
        "wv_w": (rng.standard_normal((D * HEADS, D)) * 0.02).astype(np.float32),
        "wv_b": (rng.standard_normal((D * HEADS,)) * 0.02).astype(np.float32),
        "out_w": (rng.standard_normal((D, D * HEADS)) * 0.02).astype(np.float32),
        "out_b": (rng.standard_normal((D,)) * 0.02).astype(np.float32),
    }
    o = kernel(**fake)
    print("kernel ran, out shape", o.shape, "std", o.std())
